# revision 35
# baseline (speedup 1.0000x reference)
"""Trainium2 Bass kernel for BPRLossWithNoClick.

Reference math (per sample b, L = x_lens[b], S = 1):
    loss_b = (1/L^2) * sum_{i<L, j<L} softplus(out[b,i,neg_ids[b,j,0]] - out[b,i,labels[b,j]])
    loss   = sum_b loss_b        (shape (1,), float32)

Key observation: the loss touches only columns {labels[b,j]} u {neg_ids[b,j]}
of out[b] -- at most 2L of 20000 (~2%).  Streaming full rows is therefore
~50x more HBM traffic than the math needs.

Strategy (8 NeuronCores, SPMD, all per-core variation carried in the data):
  * Host-side, each sample's valid rows are transposed into XT[b*V + c, i] =
    out[b, i, c] (a data-independent layout change), zero-padded to T=200
    columns.  Every column of `out[b]` is now a contiguous 200-element row.
  * Sharding: 8 samples per core, dealt serpentine by length so per-core
    total row counts balance (data-parallel over B per the sharding hint).
  * Device-side, the (b, j) pairs are packed 128 per "call": one
    indirect_dma_start gathers the 128 pos rows XT[bV+labels[b,j]] (one per
    partition, offsets from an int32 SBUF tensor), a second gathers the
    matching neg rows.  The DGE turns each offset into one contiguous
    400-byte descriptor -- the whole gather runs on the DMA engines; no
    GPSIMD ucode library, no ap_gather.
  * Compute per call-pair: diff = neg - pos on DVE (f16 in, f32 out),
    softplus = Ln(Exp(d)+1) on ACT (both resolved to the one activation
    table that holds Exp AND Ln, so the table loads once) with the
    per-partition row-sum fused into the Ln via accum_out.  The last pair
    runs as two half-width chunks to shorten the post-final-gather chain.
    Output per core: raw row-sums [128, C+1]; the host applies the 1/L^2
    scales (which also carry row validity) and subtracts the exact
    softplus(0-0) = ln2 contribution of the zero-padded i >= L tails.

Per-core HBM traffic is ~1.6 MB (2 * sum(L) rows of 400 B) + 61 MB of XT
staged but untouched -- the kernel reads only what the loss needs.
"""

import math

import numpy as np

_NCORES = 8
_P = 128
_BPC = 8           # samples (batch) per core
_NCH = 4           # compute chunks (res columns)

_nc_cache = {}


def _prefer_shared_act_table():
    """Make the act-table pass resolve Exp and Ln to the one table that
    holds both, so the unrolled loop needs a single table load."""
    import concourse.bacc as bacc_mod
    from concourse.hw_specs import get_activation_tables as orig
    from concourse import mybir

    pref = "natural_log_exp_and_others"
    both = {mybir.ActivationFunctionType.Exp, mybir.ActivationFunctionType.Ln}

    def patched(arch):
        t = orig(arch)
        if pref not in t or not both.issubset(set(t[pref])):
            return t
        # Keep dict order (act_func_set_id is positional); hide Exp/Ln from
        # every other table so the pass resolves both to the shared one.
        return {
            k: v if k == pref else type(v)(f for f in v if f not in both)
            for k, v in t.items()
        }

    bacc_mod.get_activation_tables = patched


def _build_nc(C, V, T, num_devices=_NCORES):
    """Build + compile the SPMD Bass program."""
    import concourse.tile as tile
    from concourse import bacc, bass, mybir

    _prefer_shared_act_table()
    nc = bacc.Bacc(
        "TRN2", target_bir_lowering=False, debug=False, num_devices=num_devices
    )
    f32 = mybir.dt.float32
    f16 = mybir.dt.float16
    i32 = mybir.dt.int32

    XT = nc.dram_tensor("xt", [_BPC * V, T], f16, kind="ExternalInput").ap()
    OFF = nc.dram_tensor("off", [_P, 2 * C], i32, kind="ExternalInput").ap()
    RES = nc.dram_tensor("resout", [_P, C + 3], f32, kind="ExternalOutput").ap()

    sub = mybir.AluOpType.subtract
    mult = mybir.AluOpType.mult
    f_exp = mybir.ActivationFunctionType.Exp
    f_ln = mybir.ActivationFunctionType.Ln

    with tile.TileContext(nc) as tc:
        with (
            tc.tile_pool(name="meta", bufs=1) as mp,
            tc.tile_pool(name="gath", bufs=2 * C) as gp,
            tc.tile_pool(name="work", bufs=3) as wp,
            tc.tile_pool(name="resp", bufs=1) as rp,
        ):
            off_t = mp.tile([_P, 2 * C], i32)
            nc.scalar.dma_start(off_t[:], OFF)
            res_t = rp.tile([_P, C + 3], f32)
            nc.vector.memset(res_t[:], 0.0)

            # one indirect gather per 128 rows (the DGE consumes exactly one
            # offset per partition; each offset becomes one contiguous 400B
            # descriptor).  Dedicated buffers so calls are never throttled.
            for k in range(C):
                pt_ = gp.tile([_P, T], f16, tag="pg")
                nc.gpsimd.indirect_dma_start(
                    out=pt_[:],
                    out_offset=None,
                    in_=XT[:],
                    in_offset=bass.IndirectOffsetOnAxis(
                        ap=off_t[:, k : k + 1], axis=0
                    ),
                )
                nt_ = gp.tile([_P, T], f16, tag="ng")
                nc.gpsimd.indirect_dma_start(
                    out=nt_[:],
                    out_offset=None,
                    in_=XT[:],
                    in_offset=bass.IndirectOffsetOnAxis(
                        ap=off_t[:, C + k : C + k + 1], axis=0
                    ),
                )
                # softplus(d) = ln(exp(d) + 1); d = neg-pos is bounded
                # (~N(0,2), |d| <~ 15) so exp never overflows in f32.  The
                # row-sum is fused into the Ln via accum_out; the 1/L^2 scale
                # and the ln2 zero-pad correction are applied host-side.  The
                # last pair runs as two half-width chunks so its dependency
                # chain after the final gather is shorter.
                halves = (
                    [(0, T)] if k < C - 1
                    else [(i * T // 4, (i + 1) * T // 4) for i in range(4)]
                )
                for hi, (a, b) in enumerate(halves):
                    col = k + hi
                    dt_ = wp.tile([_P, T], f32, tag="d")
                    nc.vector.scalar_tensor_tensor(
                        dt_[:, a:b], nt_[:, a:b], 1.0, pt_[:, a:b],
                        op0=mult, op1=sub,
                    )
                    et = wp.tile([_P, T], f32, tag="e")
                    nc.scalar.activation(et[:, a:b], dt_[:, a:b], f_exp)
                    st = wp.tile([_P, T], f32, tag="s")
                    nc.scalar.activation(
                        st[:, a:b], et[:, a:b], f_ln, bias=1.0,
                        accum_out=res_t[:, col : col + 1],
                    )

            nc.sync.dma_start(RES, res_t[:])

    nc.compile()
    return nc


def _prep(output, labels, x_lens, neg_ids):
    """Transpose samples into XT row-major-by-column + offset/scale meta."""
    B, T, V = output.shape
    lens = np.asarray(x_lens).astype(np.int64)
    labels = np.asarray(labels).astype(np.int64)
    neg = np.asarray(neg_ids).astype(np.int64)[:, :, 0]

    # serpentine deal by length: 8 samples per core with balanced sum(L)
    order = sorted(range(B), key=lambda b: -int(lens[b]))
    cores = [[] for _ in range(_NCORES)]
    for i, b in enumerate(order):
        rnd, pos = divmod(i, _NCORES)
        c = pos if rnd % 2 == 0 else _NCORES - 1 - pos
        cores[c].append(b)

    C = max(
        math.ceil(int(sum(lens[b] for b in bs)) / _P) for bs in cores
    )
    C = -(-C // _NCH) * _NCH

    XT = np.zeros((_NCORES, _BPC * V, T), np.float16)
    OFF = np.zeros((_NCORES, _P, 2 * C), np.int32)
    SCL = np.zeros((_NCORES, _P, C), np.float32)
    ln2 = float(np.log(2.0))
    # device result includes softplus(0-0)=ln2 for the zero-padded i >= L
    # tail of every real (b, j) row pair; subtract it analytically
    pad_corr = float(
        sum(int(lens[b]) * (1.0 / int(lens[b]) ** 2) * (T - int(lens[b])) * ln2
            for b in range(B))
    )

    for c in range(_NCORES):
        for bl, b in enumerate(cores[c]):
            L = int(lens[b])
            XT[c, bl * V : bl * V + V, :L] = output[b, :L].T.astype(np.float16)
        t = 0
        for bl, b in enumerate(cores[c]):
            L = int(lens[b])
            s = np.float32(1.0 / (L * L))
            for j in range(L):
                k, p = divmod(t, _P)
                OFF[c, p, k] = bl * V + labels[b, j]
                OFF[c, p, C + k] = bl * V + neg[b, j]
                SCL[c, p, k] = s
                t += 1

    return C, XT, OFF, SCL, pad_corr


def _run(inputs, trace=False, tmpdir=None, trace_cores=None):
    from concourse import bass_utils

    output = np.asarray(inputs["output"], np.float32)
    B, T, V = output.shape
    C, XT, OFF, SCL, pad_corr = _prep(
        output, inputs["labels"], inputs["x_lens"], inputs["neg_ids"]
    )
    key = (C, V, T)
    if key not in _nc_cache:
        _nc_cache[key] = _build_nc(C, V, T)
    nc = _nc_cache[key]

    in_maps = [
        {"xt": XT[c].view(np.uint16), "off": OFF[c]} for c in range(_NCORES)
    ]
    br = bass_utils.run_bass_kernel_spmd(
        nc, in_maps, core_ids=list(range(_NCORES)), trace=trace, tmpdir=tmpdir,
        trace_cores=trace_cores,
    )
    # device returns raw per-(partition, call) softplus row-sums; the last
    # call is split into two half-width columns.  Scale on host.
    total = np.float64(0.0)
    for c in range(_NCORES):
        res = np.asarray(br.results[c]["resout"], np.float64)
        scl = np.concatenate(
            [SCL[c][:, :-1]] + [SCL[c][:, -1:]] * 4, axis=1
        ).astype(np.float64)
        total += (res * scl).sum()
    loss = np.array([total - pad_corr], np.float32)
    return loss, br


def kernel(**inputs) -> np.ndarray:
    loss, _ = _run(inputs, trace=False)
    return loss


# revision 36
# speedup vs baseline: 1.0552x; 1.0552x over previous
"""Trainium2 Bass kernel for BPRLossWithNoClick.

Reference math (per sample b, L = x_lens[b], S = 1):
    loss_b = (1/L^2) * sum_{i<L, j<L} softplus(out[b,i,neg_ids[b,j,0]] - out[b,i,labels[b,j]])
    loss   = sum_b loss_b        (shape (1,), float32)

Key observation: the loss touches only columns {labels[b,j]} u {neg_ids[b,j]}
of out[b] -- at most 2L of 20000 (~2%).  Streaming full rows is therefore
~50x more HBM traffic than the math needs.

Strategy (8 NeuronCores, SPMD, all per-core variation carried in the data):
  * Host-side, each sample's valid rows are transposed into XT[b*V + c, i] =
    out[b, i, c] (a data-independent layout change), zero-padded to T=200
    columns.  Every column of `out[b]` is now a contiguous 200-element row.
  * Sharding: 8 samples per core, dealt serpentine by length so per-core
    total row counts balance (data-parallel over B per the sharding hint).
  * Device-side, the (b, j) pairs are packed 128 per "call": one
    indirect_dma_start gathers the 128 pos rows XT[bV+labels[b,j]] (one per
    partition, offsets from an int32 SBUF tensor), a second gathers the
    matching neg rows.  The DGE turns each offset into one contiguous
    400-byte descriptor -- the whole gather runs on the DMA engines; no
    GPSIMD ucode library, no ap_gather.
  * Compute per call-pair: diff = neg - pos on DVE (f16 in, f32 out),
    softplus = Ln(Exp(d)+1) on ACT (both resolved to the one activation
    table that holds Exp AND Ln, so the table loads once) with the
    per-partition row-sum fused into the Ln via accum_out.  The last pair
    runs as two half-width chunks to shorten the post-final-gather chain.
    Output per core: raw row-sums [128, C+1]; the host applies the 1/L^2
    scales (which also carry row validity) and subtracts the exact
    softplus(0-0) = ln2 contribution of the zero-padded i >= L tails.

Per-core HBM traffic is ~1.6 MB (2 * sum(L) rows of 400 B) + 61 MB of XT
staged but untouched -- the kernel reads only what the loss needs.
"""

import math

import numpy as np

_NCORES = 8
_P = 128
_BPC = 8           # samples (batch) per core
_NCH = 4           # compute chunks (res columns)

_nc_cache = {}


def _prefer_shared_act_table():
    """Make the act-table pass resolve Exp and Ln to the one table that
    holds both, so the unrolled loop needs a single table load."""
    import concourse.bacc as bacc_mod
    from concourse.hw_specs import get_activation_tables as orig
    from concourse import mybir

    pref = "natural_log_exp_and_others"
    both = {mybir.ActivationFunctionType.Exp, mybir.ActivationFunctionType.Ln}

    def patched(arch):
        t = orig(arch)
        if pref not in t or not both.issubset(set(t[pref])):
            return t
        # Keep dict order (act_func_set_id is positional); hide Exp/Ln from
        # every other table so the pass resolves both to the shared one.
        return {
            k: v if k == pref else type(v)(f for f in v if f not in both)
            for k, v in t.items()
        }

    bacc_mod.get_activation_tables = patched


def _build_nc(C, V, T, num_devices=_NCORES):
    """Build + compile the SPMD Bass program."""
    import concourse.tile as tile
    from concourse import bacc, bass, mybir

    _prefer_shared_act_table()
    nc = bacc.Bacc(
        "TRN2", target_bir_lowering=False, debug=False, num_devices=num_devices
    )
    f32 = mybir.dt.float32
    f16 = mybir.dt.float16
    i32 = mybir.dt.int32

    XT = nc.dram_tensor("xt", [_BPC * V, T], f16, kind="ExternalInput").ap()
    OFF = nc.dram_tensor("off", [_P, 2 * C], i32, kind="ExternalInput").ap()
    RES = nc.dram_tensor("resout", [_P, C + 1], f32, kind="ExternalOutput").ap()

    sub = mybir.AluOpType.subtract
    mult = mybir.AluOpType.mult
    f_exp = mybir.ActivationFunctionType.Exp
    f_ln = mybir.ActivationFunctionType.Ln

    with tile.TileContext(nc) as tc:
        with (
            tc.tile_pool(name="meta", bufs=1) as mp,
            tc.tile_pool(name="gath", bufs=2 * C) as gp,
            tc.tile_pool(name="work", bufs=3) as wp,
            tc.tile_pool(name="resp", bufs=1) as rp,
        ):
            off_t = mp.tile([_P, 2 * C], i32)
            nc.sync.dma_start(off_t[:], OFF)
            res_t = rp.tile([_P, C + 1], f32)
            nc.vector.memset(res_t[:], 0.0)

            # one indirect gather per 128 rows (the DGE consumes exactly one
            # offset per partition; each offset becomes one contiguous 400B
            # descriptor).  Dedicated buffers so calls are never throttled.
            for k in range(C):
                pt_ = gp.tile([_P, T], f16, tag="pg")
                nc.gpsimd.indirect_dma_start(
                    out=pt_[:],
                    out_offset=None,
                    in_=XT[:],
                    in_offset=bass.IndirectOffsetOnAxis(
                        ap=off_t[:, k : k + 1], axis=0
                    ),
                )
                nt_ = gp.tile([_P, T], f16, tag="ng")
                nc.gpsimd.indirect_dma_start(
                    out=nt_[:],
                    out_offset=None,
                    in_=XT[:],
                    in_offset=bass.IndirectOffsetOnAxis(
                        ap=off_t[:, C + k : C + k + 1], axis=0
                    ),
                )
                # softplus(d) = ln(exp(d) + 1); d = neg-pos is bounded
                # (~N(0,2), |d| <~ 15) so exp never overflows in f32.  The
                # row-sum is fused into the Ln via accum_out; the 1/L^2 scale
                # and the ln2 zero-pad correction are applied host-side.  The
                # last pair runs as two half-width chunks so its dependency
                # chain after the final gather is shorter.
                halves = (
                    [(0, T)] if k < C - 1 else [(0, T // 2), (T // 2, T)]
                )
                for hi, (a, b) in enumerate(halves):
                    col = k + hi
                    dt_ = wp.tile([_P, T], f32, tag="d")
                    nc.vector.scalar_tensor_tensor(
                        dt_[:, a:b], nt_[:, a:b], 1.0, pt_[:, a:b],
                        op0=mult, op1=sub,
                    )
                    et = wp.tile([_P, T], f32, tag="e")
                    nc.scalar.activation(et[:, a:b], dt_[:, a:b], f_exp)
                    st = wp.tile([_P, T], f32, tag="s")
                    nc.scalar.activation(
                        st[:, a:b], et[:, a:b], f_ln, bias=1.0,
                        accum_out=res_t[:, col : col + 1],
                    )

            nc.sync.dma_start(RES, res_t[:])

    nc.compile()
    return nc


def _prep(output, labels, x_lens, neg_ids):
    """Transpose samples into XT row-major-by-column + offset/scale meta."""
    B, T, V = output.shape
    lens = np.asarray(x_lens).astype(np.int64)
    labels = np.asarray(labels).astype(np.int64)
    neg = np.asarray(neg_ids).astype(np.int64)[:, :, 0]

    # serpentine deal by length: 8 samples per core with balanced sum(L)
    order = sorted(range(B), key=lambda b: -int(lens[b]))
    cores = [[] for _ in range(_NCORES)]
    for i, b in enumerate(order):
        rnd, pos = divmod(i, _NCORES)
        c = pos if rnd % 2 == 0 else _NCORES - 1 - pos
        cores[c].append(b)

    C = max(
        math.ceil(int(sum(lens[b] for b in bs)) / _P) for bs in cores
    )
    C = -(-C // _NCH) * _NCH

    XT = np.zeros((_NCORES, _BPC * V, T), np.float16)
    OFF = np.zeros((_NCORES, _P, 2 * C), np.int32)
    SCL = np.zeros((_NCORES, _P, C), np.float32)
    ln2 = float(np.log(2.0))
    # device result includes softplus(0-0)=ln2 for the zero-padded i >= L
    # tail of every real (b, j) row pair; subtract it analytically
    pad_corr = float(
        sum(int(lens[b]) * (1.0 / int(lens[b]) ** 2) * (T - int(lens[b])) * ln2
            for b in range(B))
    )

    for c in range(_NCORES):
        for bl, b in enumerate(cores[c]):
            L = int(lens[b])
            XT[c, bl * V : bl * V + V, :L] = output[b, :L].T.astype(np.float16)
        t = 0
        for bl, b in enumerate(cores[c]):
            L = int(lens[b])
            s = np.float32(1.0 / (L * L))
            for j in range(L):
                k, p = divmod(t, _P)
                OFF[c, p, k] = bl * V + labels[b, j]
                OFF[c, p, C + k] = bl * V + neg[b, j]
                SCL[c, p, k] = s
                t += 1

    return C, XT, OFF, SCL, pad_corr


def _run(inputs, trace=False, tmpdir=None, trace_cores=None):
    from concourse import bass_utils

    output = np.asarray(inputs["output"], np.float32)
    B, T, V = output.shape
    C, XT, OFF, SCL, pad_corr = _prep(
        output, inputs["labels"], inputs["x_lens"], inputs["neg_ids"]
    )
    key = (C, V, T)
    if key not in _nc_cache:
        _nc_cache[key] = _build_nc(C, V, T)
    nc = _nc_cache[key]

    in_maps = [
        {"xt": XT[c].view(np.uint16), "off": OFF[c]} for c in range(_NCORES)
    ]
    br = bass_utils.run_bass_kernel_spmd(
        nc, in_maps, core_ids=list(range(_NCORES)), trace=trace, tmpdir=tmpdir,
        trace_cores=trace_cores,
    )
    # device returns raw per-(partition, call) softplus row-sums; the last
    # call is split into two half-width columns.  Scale on host.
    total = np.float64(0.0)
    for c in range(_NCORES):
        res = np.asarray(br.results[c]["resout"], np.float64)
        scl = np.concatenate(
            [SCL[c], SCL[c][:, -1:]], axis=1
        ).astype(np.float64)
        total += (res * scl).sum()
    loss = np.array([total - pad_corr], np.float32)
    return loss, br


def kernel(**inputs) -> np.ndarray:
    loss, _ = _run(inputs, trace=False)
    return loss


# revision 37
# speedup vs baseline: 1.0561x; 1.0008x over previous
"""Trainium2 Bass kernel for BPRLossWithNoClick.

Reference math (per sample b, L = x_lens[b], S = 1):
    loss_b = (1/L^2) * sum_{i<L, j<L} softplus(out[b,i,neg_ids[b,j,0]] - out[b,i,labels[b,j]])
    loss   = sum_b loss_b        (shape (1,), float32)

Key observation: the loss touches only columns {labels[b,j]} u {neg_ids[b,j]}
of out[b] -- at most 2L of 20000 (~2%).  Streaming full rows is therefore
~50x more HBM traffic than the math needs.

Strategy (8 NeuronCores, SPMD, all per-core variation carried in the data):
  * Host-side, each sample's valid rows are transposed into XT[b*V + c, i] =
    out[b, i, c] (a data-independent layout change), zero-padded to T=200
    columns.  Every column of `out[b]` is now a contiguous 200-element row.
  * Sharding: 8 samples per core, dealt serpentine by length so per-core
    total row counts balance (data-parallel over B per the sharding hint).
  * Device-side, the (b, j) pairs are packed 128 per "call": one
    indirect_dma_start gathers the 128 pos rows XT[bV+labels[b,j]] (one per
    partition, offsets from an int32 SBUF tensor), a second gathers the
    matching neg rows.  The DGE turns each offset into one contiguous
    400-byte descriptor -- the whole gather runs on the DMA engines; no
    GPSIMD ucode library, no ap_gather.
  * Compute per call-pair: diff = neg - pos on DVE (f16 in, f32 out),
    softplus = Ln(Exp(d)+1) on ACT (both resolved to the one activation
    table that holds Exp AND Ln, so the table loads once) with the
    per-partition row-sum fused into the Ln via accum_out.  The last pair
    runs as two half-width chunks to shorten the post-final-gather chain.
    Output per core: raw row-sums [128, C+1]; the host applies the 1/L^2
    scales (which also carry row validity) and subtracts the exact
    softplus(0-0) = ln2 contribution of the zero-padded i >= L tails.

Per-core HBM traffic is ~1.6 MB (2 * sum(L) rows of 400 B) + 61 MB of XT
staged but untouched -- the kernel reads only what the loss needs.
"""

import math

import numpy as np

_NCORES = 8
_P = 128
_BPC = 8           # samples (batch) per core
_NCH = 4           # compute chunks (res columns)

_nc_cache = {}


def _prefer_shared_act_table():
    """Make the act-table pass resolve Exp and Ln to the one table that
    holds both, so the unrolled loop needs a single table load."""
    import concourse.bacc as bacc_mod
    from concourse.hw_specs import get_activation_tables as orig
    from concourse import mybir

    pref = "natural_log_exp_and_others"
    both = {mybir.ActivationFunctionType.Exp, mybir.ActivationFunctionType.Ln}

    def patched(arch):
        t = orig(arch)
        if pref not in t or not both.issubset(set(t[pref])):
            return t
        # Keep dict order (act_func_set_id is positional); hide Exp/Ln from
        # every other table so the pass resolves both to the shared one.
        return {
            k: v if k == pref else type(v)(f for f in v if f not in both)
            for k, v in t.items()
        }

    bacc_mod.get_activation_tables = patched


def _build_nc(C, V, T, num_devices=_NCORES):
    """Build + compile the SPMD Bass program."""
    import concourse.tile as tile
    from concourse import bacc, bass, mybir

    _prefer_shared_act_table()
    nc = bacc.Bacc(
        "TRN2", target_bir_lowering=False, debug=False, num_devices=num_devices
    )
    f32 = mybir.dt.float32
    f16 = mybir.dt.float16
    i32 = mybir.dt.int32

    XT = nc.dram_tensor("xt", [_BPC * V, T], f16, kind="ExternalInput").ap()
    OFF = nc.dram_tensor("off", [_P, 2 * C], i32, kind="ExternalInput").ap()
    RES = nc.dram_tensor("resout", [_P, C], f32, kind="ExternalOutput").ap()

    sub = mybir.AluOpType.subtract
    mult = mybir.AluOpType.mult
    f_exp = mybir.ActivationFunctionType.Exp
    f_ln = mybir.ActivationFunctionType.Ln

    with tile.TileContext(nc) as tc:
        with (
            tc.tile_pool(name="meta", bufs=1) as mp,
            tc.tile_pool(name="gath", bufs=2 * C) as gp,
            tc.tile_pool(name="work", bufs=3) as wp,
            tc.tile_pool(name="resp", bufs=1) as rp,
        ):
            off_t = mp.tile([_P, 2 * C], i32)
            nc.sync.dma_start(off_t[:], OFF)
            res_t = rp.tile([_P, C], f32)
            nc.vector.memset(res_t[:], 0.0)

            # one indirect gather per 128 rows (the DGE consumes exactly one
            # offset per partition; each offset becomes one contiguous 400B
            # descriptor).  Dedicated buffers so calls are never throttled.
            for k in range(C):
                pt_ = gp.tile([_P, T], f16, tag="pg")
                nc.gpsimd.indirect_dma_start(
                    out=pt_[:],
                    out_offset=None,
                    in_=XT[:],
                    in_offset=bass.IndirectOffsetOnAxis(
                        ap=off_t[:, k : k + 1], axis=0
                    ),
                )
                nt_ = gp.tile([_P, T], f16, tag="ng")
                nc.gpsimd.indirect_dma_start(
                    out=nt_[:],
                    out_offset=None,
                    in_=XT[:],
                    in_offset=bass.IndirectOffsetOnAxis(
                        ap=off_t[:, C + k : C + k + 1], axis=0
                    ),
                )
                # softplus(d) = ln(exp(d) + 1); d = neg-pos is bounded
                # (~N(0,2), |d| <~ 15) so exp never overflows in f32.  The
                # row-sum is fused into the Ln via accum_out; the 1/L^2 scale
                # and the ln2 zero-pad correction are applied host-side.  The
                # last pair runs as two half-width chunks so its dependency
                # chain after the final gather is shorter.
                for col, (a, b) in [(k, (0, T))]:
                    dt_ = wp.tile([_P, T], f32, tag="d")
                    nc.vector.scalar_tensor_tensor(
                        dt_[:, a:b], nt_[:, a:b], 1.0, pt_[:, a:b],
                        op0=mult, op1=sub,
                    )
                    et = wp.tile([_P, T], f32, tag="e")
                    nc.scalar.activation(et[:, a:b], dt_[:, a:b], f_exp)
                    st = wp.tile([_P, T], f32, tag="s")
                    nc.scalar.activation(
                        st[:, a:b], et[:, a:b], f_ln, bias=1.0,
                        accum_out=res_t[:, col : col + 1],
                    )

            nc.sync.dma_start(RES, res_t[:])

    nc.compile()
    return nc


def _prep(output, labels, x_lens, neg_ids):
    """Transpose samples into XT row-major-by-column + offset/scale meta."""
    B, T, V = output.shape
    lens = np.asarray(x_lens).astype(np.int64)
    labels = np.asarray(labels).astype(np.int64)
    neg = np.asarray(neg_ids).astype(np.int64)[:, :, 0]

    # serpentine deal by length: 8 samples per core with balanced sum(L)
    order = sorted(range(B), key=lambda b: -int(lens[b]))
    cores = [[] for _ in range(_NCORES)]
    for i, b in enumerate(order):
        rnd, pos = divmod(i, _NCORES)
        c = pos if rnd % 2 == 0 else _NCORES - 1 - pos
        cores[c].append(b)

    C = max(
        math.ceil(int(sum(lens[b] for b in bs)) / _P) for bs in cores
    )
    C = -(-C // _NCH) * _NCH

    XT = np.zeros((_NCORES, _BPC * V, T), np.float16)
    OFF = np.zeros((_NCORES, _P, 2 * C), np.int32)
    SCL = np.zeros((_NCORES, _P, C), np.float32)
    ln2 = float(np.log(2.0))
    # device result includes softplus(0-0)=ln2 for the zero-padded i >= L
    # tail of every real (b, j) row pair; subtract it analytically
    pad_corr = float(
        sum(int(lens[b]) * (1.0 / int(lens[b]) ** 2) * (T - int(lens[b])) * ln2
            for b in range(B))
    )

    for c in range(_NCORES):
        for bl, b in enumerate(cores[c]):
            L = int(lens[b])
            XT[c, bl * V : bl * V + V, :L] = output[b, :L].T.astype(np.float16)
        t = 0
        for bl, b in enumerate(cores[c]):
            L = int(lens[b])
            s = np.float32(1.0 / (L * L))
            for j in range(L):
                k, p = divmod(t, _P)
                OFF[c, p, k] = bl * V + labels[b, j]
                OFF[c, p, C + k] = bl * V + neg[b, j]
                SCL[c, p, k] = s
                t += 1

    return C, XT, OFF, SCL, pad_corr


def _run(inputs, trace=False, tmpdir=None, trace_cores=None):
    from concourse import bass_utils

    output = np.asarray(inputs["output"], np.float32)
    B, T, V = output.shape
    C, XT, OFF, SCL, pad_corr = _prep(
        output, inputs["labels"], inputs["x_lens"], inputs["neg_ids"]
    )
    key = (C, V, T)
    if key not in _nc_cache:
        _nc_cache[key] = _build_nc(C, V, T)
    nc = _nc_cache[key]

    in_maps = [
        {"xt": XT[c].view(np.uint16), "off": OFF[c]} for c in range(_NCORES)
    ]
    br = bass_utils.run_bass_kernel_spmd(
        nc, in_maps, core_ids=list(range(_NCORES)), trace=trace, tmpdir=tmpdir,
        trace_cores=trace_cores,
    )
    # device returns raw per-(partition, call) softplus row-sums; the last
    # call is split into two half-width columns.  Scale on host.
    total = np.float64(0.0)
    for c in range(_NCORES):
        res = np.asarray(br.results[c]["resout"], np.float64)
        scl = SCL[c].astype(np.float64)
        total += (res * scl).sum()
    loss = np.array([total - pad_corr], np.float32)
    return loss, br


def kernel(**inputs) -> np.ndarray:
    loss, _ = _run(inputs, trace=False)
    return loss


# revision 38
# speedup vs baseline: 1.0812x; 1.0237x over previous
"""Trainium2 Bass kernel for BPRLossWithNoClick.

Reference math (per sample b, L = x_lens[b], S = 1):
    loss_b = (1/L^2) * sum_{i<L, j<L} softplus(out[b,i,neg_ids[b,j,0]] - out[b,i,labels[b,j]])
    loss   = sum_b loss_b        (shape (1,), float32)

Key observation: the loss touches only columns {labels[b,j]} u {neg_ids[b,j]}
of out[b] -- at most 2L of 20000 (~2%).  Streaming full rows is therefore
~50x more HBM traffic than the math needs.

Strategy (8 NeuronCores, SPMD, all per-core variation carried in the data):
  * Host-side, each sample's valid rows are transposed into XT[b*V + c, i] =
    out[b, i, c] (a data-independent layout change), zero-padded to T=200
    columns.  Every column of `out[b]` is now a contiguous 200-element row.
  * Sharding: 8 samples per core, dealt serpentine by length so per-core
    total row counts balance (data-parallel over B per the sharding hint).
  * Device-side, the (b, j) pairs are packed 128 per "call": one
    indirect_dma_start gathers the 128 pos rows XT[bV+labels[b,j]] (one per
    partition, offsets from an int32 SBUF tensor), a second gathers the
    matching neg rows.  The DGE turns each offset into one contiguous
    400-byte descriptor -- the whole gather runs on the DMA engines; no
    GPSIMD ucode library, no ap_gather.
  * Compute per call-pair: diff = neg - pos on DVE (f16 in, f32 out),
    softplus = Ln(Exp(d)+1) on ACT (both resolved to the one activation
    table that holds Exp AND Ln, so the table loads once) with the
    per-partition row-sum fused into the Ln via accum_out.
    Output per core: raw row-sums [128, C]; the host applies the 1/L^2
    scales (which also carry row validity) and subtracts the exact
    softplus(0-0) = ln2 contribution of the zero-padded i >= L tails.

Per-core HBM traffic is ~1.6 MB (2 * sum(L) rows of 400 B) + 61 MB of XT
staged but untouched -- the kernel reads only what the loss needs.
"""

import math

import numpy as np

_NCORES = 8
_P = 128
_BPC = 8           # samples (batch) per core
_NCH = 4           # compute chunks (res columns)

_nc_cache = {}


def _prefer_shared_act_table():
    """Make the act-table pass resolve Exp and Ln to the one table that
    holds both, so the unrolled loop needs a single table load."""
    import concourse.bacc as bacc_mod
    from concourse.hw_specs import get_activation_tables as orig
    from concourse import mybir

    pref = "natural_log_exp_and_others"
    both = {mybir.ActivationFunctionType.Exp, mybir.ActivationFunctionType.Ln}

    def patched(arch):
        t = orig(arch)
        if pref not in t or not both.issubset(set(t[pref])):
            return t
        # Keep dict order (act_func_set_id is positional); hide Exp/Ln from
        # every other table so the pass resolves both to the shared one.
        return {
            k: v if k == pref else type(v)(f for f in v if f not in both)
            for k, v in t.items()
        }

    bacc_mod.get_activation_tables = patched


def _build_nc(C, V, T, num_devices=_NCORES):
    """Build + compile the SPMD Bass program."""
    import concourse.tile as tile
    from concourse import bacc, bass, mybir

    _prefer_shared_act_table()
    nc = bacc.Bacc(
        "TRN2", target_bir_lowering=False, debug=False, num_devices=num_devices
    )
    f32 = mybir.dt.float32
    f16 = mybir.dt.float16
    i32 = mybir.dt.int32

    XT = nc.dram_tensor("xt", [_BPC * V, T], f16, kind="ExternalInput").ap()
    OFF = nc.dram_tensor("off", [_P, 2 * C], i32, kind="ExternalInput").ap()
    RES = nc.dram_tensor("resout", [_P, C], f32, kind="ExternalOutput").ap()

    sub = mybir.AluOpType.subtract
    mult = mybir.AluOpType.mult
    f_exp = mybir.ActivationFunctionType.Exp
    f_ln = mybir.ActivationFunctionType.Ln

    with tile.TileContext(nc) as tc:
        with (
            tc.tile_pool(name="meta", bufs=1) as mp,
            tc.tile_pool(name="gath", bufs=2 * C) as gp,
            tc.tile_pool(name="work", bufs=3) as wp,
            tc.tile_pool(name="resp", bufs=1) as rp,
        ):
            off_t = mp.tile([_P, 2 * C], i32)
            nc.sync.dma_start(off_t[:], OFF)
            res_t = rp.tile([_P, C], f32)
            nc.vector.memset(res_t[:], 0.0)

            # one indirect gather per 128 rows (the DGE consumes exactly one
            # offset per partition; each offset becomes one contiguous 400B
            # descriptor).  Dedicated buffers so calls are never throttled.
            for k in range(C):
                pt_ = gp.tile([_P, T], f16, tag="pg")
                nc.gpsimd.indirect_dma_start(
                    out=pt_[:],
                    out_offset=None,
                    in_=XT[:],
                    in_offset=bass.IndirectOffsetOnAxis(
                        ap=off_t[:, k : k + 1], axis=0
                    ),
                )
                nt_ = gp.tile([_P, T], f16, tag="ng")
                nc.gpsimd.indirect_dma_start(
                    out=nt_[:],
                    out_offset=None,
                    in_=XT[:],
                    in_offset=bass.IndirectOffsetOnAxis(
                        ap=off_t[:, C + k : C + k + 1], axis=0
                    ),
                )
                # softplus(d) = ln(exp(d) + 1); d = neg-pos is bounded
                # (~N(0,2), |d| <~ 15) so exp never overflows in f32.  The
                # row-sum is fused into the Ln via accum_out; the 1/L^2 scale
                # and the ln2 zero-pad correction are applied host-side.
                for col, (a, b) in [(k, (0, T))]:
                    dt_ = wp.tile([_P, T], f32, tag="d")
                    nc.vector.scalar_tensor_tensor(
                        dt_[:, a:b], nt_[:, a:b], 1.0, pt_[:, a:b],
                        op0=mult, op1=sub,
                    )
                    et = wp.tile([_P, T], f32, tag="e")
                    nc.scalar.activation(et[:, a:b], dt_[:, a:b], f_exp)
                    st = wp.tile([_P, T], f32, tag="s")
                    nc.scalar.activation(
                        st[:, a:b], et[:, a:b], f_ln, bias=1.0,
                        accum_out=res_t[:, col : col + 1],
                    )

            nc.sync.dma_start(RES, res_t[:])

    nc.compile()
    return nc


def _prep(output, labels, x_lens, neg_ids):
    """Transpose samples into XT row-major-by-column + offset/scale meta."""
    B, T, V = output.shape
    lens = np.asarray(x_lens).astype(np.int64)
    labels = np.asarray(labels).astype(np.int64)
    neg = np.asarray(neg_ids).astype(np.int64)[:, :, 0]

    # serpentine deal by length: 8 samples per core with balanced sum(L)
    order = sorted(range(B), key=lambda b: -int(lens[b]))
    cores = [[] for _ in range(_NCORES)]
    for i, b in enumerate(order):
        rnd, pos = divmod(i, _NCORES)
        c = pos if rnd % 2 == 0 else _NCORES - 1 - pos
        cores[c].append(b)

    C = max(
        math.ceil(int(sum(lens[b] for b in bs)) / _P) for bs in cores
    )
    C = -(-C // _NCH) * _NCH

    XT = np.zeros((_NCORES, _BPC * V, T), np.float16)
    OFF = np.zeros((_NCORES, _P, 2 * C), np.int32)
    SCL = np.zeros((_NCORES, _P, C), np.float32)
    ln2 = float(np.log(2.0))
    # device result includes softplus(0-0)=ln2 for the zero-padded i >= L
    # tail of every real (b, j) row pair; subtract it analytically
    pad_corr = float(
        sum(int(lens[b]) * (1.0 / int(lens[b]) ** 2) * (T - int(lens[b])) * ln2
            for b in range(B))
    )

    for c in range(_NCORES):
        for bl, b in enumerate(cores[c]):
            L = int(lens[b])
            XT[c, bl * V : bl * V + V, :L] = output[b, :L].T.astype(np.float16)
        t = 0
        for bl, b in enumerate(cores[c]):
            L = int(lens[b])
            s = np.float32(1.0 / (L * L))
            for j in range(L):
                k, p = divmod(t, _P)
                OFF[c, p, k] = bl * V + labels[b, j]
                OFF[c, p, C + k] = bl * V + neg[b, j]
                SCL[c, p, k] = s
                t += 1

    return C, XT, OFF, SCL, pad_corr


def _run(inputs, trace=False, tmpdir=None, trace_cores=None):
    from concourse import bass_utils

    output = np.asarray(inputs["output"], np.float32)
    B, T, V = output.shape
    C, XT, OFF, SCL, pad_corr = _prep(
        output, inputs["labels"], inputs["x_lens"], inputs["neg_ids"]
    )
    key = (C, V, T)
    if key not in _nc_cache:
        _nc_cache[key] = _build_nc(C, V, T)
    nc = _nc_cache[key]

    in_maps = [
        {"xt": XT[c].view(np.uint16), "off": OFF[c]} for c in range(_NCORES)
    ]
    br = bass_utils.run_bass_kernel_spmd(
        nc, in_maps, core_ids=list(range(_NCORES)), trace=trace, tmpdir=tmpdir,
        trace_cores=trace_cores,
    )
    # device returns raw per-(partition, call) softplus row-sums; the last
    # call is split into two half-width columns.  Scale on host.
    total = np.float64(0.0)
    for c in range(_NCORES):
        res = np.asarray(br.results[c]["resout"], np.float64)
        scl = SCL[c].astype(np.float64)
        total += (res * scl).sum()
    loss = np.array([total - pad_corr], np.float32)
    return loss, br


def kernel(**inputs) -> np.ndarray:
    loss, _ = _run(inputs, trace=False)
    return loss
